# revision 3
# baseline (speedup 1.0000x reference)
"""Bundle-adjustment projection kernel v7 for Trainium2 (8 NeuronCores).

v6 reworked for the GPSIMD-cannot-touch-PSUM rule: per 2-macro group the
packed psum tiles pn=(NX0;NY0;NX1;NY1) and rec=(1/D0;1/D0;1/D1;1/D1) allow a
single [128,512] product.  Most groups route it as Act copy (PSUM->SBUF f32)
+ Pool mult (SBUF, writes f16); a few groups run the product directly on DVE.
Out tiles are [128,512] per group (DMA cost halved vs [64,1024]).
"""
import sys
sys.path.insert(0, "/opt/trn_rl_repo")

import numpy as np

FX, FY, CX, CY = 320.0, 320.0, 320.0, 240.0
N_MP, N_KF, M = 200000, 2000, 4000000
N_CORES = 8
B = 512                      # edges per block (one kf per block)
BPM = 32                     # blocks per macro-tile
SPM = B * BPM                # 16384 slots per macro
GROUP = 2                    # macros per psum-bank group (= X chunk)
N_MACRO = 34                 # macros per core
N_GRP = N_MACRO // GROUP
N_BLOCKS_CAP = N_CORES * N_MACRO * BPM
SLOTS_CORE = N_MACRO * SPM
SLOTS_TOTAL = N_CORES * SLOTS_CORE
WC = 128                     # W cols per macro (64 numer + 64 dup-denom)
XRING = 3
DVE_GROUPS = frozenset((4, 12))   # groups whose product runs on DVE

_CACHE = {}


def _build(n_rep=1):
    import concourse.bacc as bacc
    import concourse.mybir as mybir
    import concourse.tile as tile

    f32 = mybir.dt.float32
    f16 = mybir.dt.float16
    Alu = mybir.AluOpType

    nc = bacc.Bacc(None, target_bir_lowering=False)
    x_h = nc.dram_tensor("X", [96, N_MACRO * B], f16, kind="ExternalInput")
    w_h = nc.dram_tensor("W", [128, N_MACRO * WC], f16, kind="ExternalInput")
    out_h = nc.dram_tensor("out", [128, N_GRP * B], f16, kind="ExternalOutput")

    with tile.TileContext(nc) as tc:
        with (
            tc.tile_pool(name="const", bufs=1) as constp,
            tc.tile_pool(name="res", bufs=4) as resp,
            tc.tile_pool(name="psn", bufs=3, space="PSUM") as npool,
            tc.tile_pool(name="psd", bufs=3, space="PSUM") as dpool,
        ):
            wt = constp.tile([128, N_MACRO * WC], f16)
            wcols = N_MACRO * WC
            wq = wcols // 2
            nc.gpsimd.dma_start(wt[:, 0:wq], w_h[:, 0:wq])
            nc.scalar.dma_start(wt[:, wq:wcols], w_h[:, wq:wcols])
            xtiles = []
            for k in range(XRING):
                xr = constp.tile([128, GROUP * B], f16, name=f"xring{k}")
                nc.vector.memset(xr[96:128, :], 1.0)
                xtiles.append(xr)

            def _body():
                for g in range(N_GRP):
                    xc = xtiles[g % XRING]
                    nc.sync.dma_start(
                        xc[0:96, :],
                        x_h[:, g * GROUP * B:(g + 1) * GROUP * B])
                    pn = npool.tile([128, B], f32, tag="pn")
                    pd = dpool.tile([128, B], f32, tag="pd")
                    for i in range(GROUP):
                        m = g * GROUP + i
                        xs = xc[:, i * B:(i + 1) * B]
                        nc.tensor.matmul(out=pn[64 * i:64 * (i + 1), :],
                                         lhsT=wt[:, m * WC:m * WC + 64],
                                         rhs=xs, start=True, stop=True)
                        nc.tensor.matmul(out=pd[64 * i:64 * (i + 1), :],
                                         lhsT=wt[:, m * WC + 64:m * WC + 128],
                                         rhs=xs, start=True, stop=True)
                    rec = resp.tile([128, B], f32, tag="rec")
                    nc.vector.reciprocal_approx_fast(rec[:], pd[:])
                    xy = resp.tile([128, B], f16, tag="xy")
                    if g in DVE_GROUPS:
                        nc.vector.tensor_tensor(
                            xy[:, :], pn[:, :], rec[:, :], op=Alu.mult)
                    else:
                        ncp = resp.tile([128, B], f32, tag="ncp")
                        nc.scalar.copy(ncp[:], pn[:, :])
                        nc.gpsimd.tensor_tensor(
                            xy[:, :], ncp[:], rec[:, :], op=Alu.mult)
                    oeng = nc.scalar if g % 2 == 0 else nc.gpsimd
                    oeng.dma_start(out_h[:, g * B:(g + 1) * B], xy[:, :])

            if n_rep == 1:
                _body()
            else:
                with tc.For_i(0, n_rep, 1):
                    _body()
    nc.finalize()
    return nc


def _prep_inputs(tMP, tKF, kf_ids, mp_ids, idxKF, idxMP):
    tMP = np.asarray(tMP, np.float32)
    tKF = np.asarray(tKF, np.float32)
    idsKF = np.searchsorted(np.asarray(idxKF), np.asarray(kf_ids)).astype(np.int64)
    idsMP = np.searchsorted(np.asarray(idxMP), np.asarray(mp_ids)).astype(np.int64)

    order = np.argsort(idsKF, kind="stable")
    kf_s = idsKF[order]
    mp_s = idsMP[order]

    counts = np.bincount(kf_s, minlength=N_KF)
    blocks_k = -(-counts // B)          # ceil
    total_blocks = int(blocks_k.sum())
    assert total_blocks <= N_BLOCKS_CAP, (
        f"block capacity exceeded: {total_blocks} > {N_BLOCKS_CAP}")

    block_start = np.zeros(N_KF, np.int64)
    np.cumsum(blocks_k[:-1], out=block_start[1:])
    first = np.cumsum(counts) - counts
    slot = block_start[kf_s] * B + (np.arange(M) - first[kf_s])

    blk_kf = np.zeros(N_BLOCKS_CAP, np.int64)
    blk_kf[:total_blocks] = np.repeat(np.arange(N_KF), blocks_k)

    # X stream: 3 features only, partition 3b+f
    X = np.ones((SLOTS_TOTAL, 3), np.float16)
    X[slot] = tMP[mp_s].astype(np.float16)
    Xdev = np.ascontiguousarray(
        X.reshape(N_CORES, N_MACRO, BPM, B, 3)
         .transpose(0, 2, 4, 1, 3)          # core, b, f, m, j
         .reshape(N_CORES, 96, N_MACRO * B))

    T = tKF
    A = np.stack([FX * T[:, 0, :] + CX * T[:, 2, :],
                  FY * T[:, 1, :] + CY * T[:, 2, :],
                  T[:, 2, :]], axis=1)  # [N_KF, 3, 4]
    blk_A = A[blk_kf].astype(np.float16)
    # W rows: p=3b+f (f<3) -> A[b,gi,f];  p=96+b -> A[b,gi,3]
    # cols: 0:32 numerX, 32:64 numerY, 64:96 denom, 96:128 denom (dup)
    n_cm = N_BLOCKS_CAP // BPM
    W = np.zeros((n_cm, 128, WC), np.float16)
    cm = np.arange(N_BLOCKS_CAP) // BPM
    bb = np.arange(N_BLOCKS_CAP) % BPM
    for ci, gi in enumerate((0, 1, 2, 2)):
        col = 32 * ci + bb
        for f in range(3):
            W[cm, 3 * bb + f, col] = blk_A[:, gi, f]
        W[cm, 96 + bb, col] = blk_A[:, gi, 3]
    Wdev = np.ascontiguousarray(
        W.reshape(N_CORES, N_MACRO, 128, WC)
         .transpose(0, 2, 1, 3)
         .reshape(N_CORES, 128, N_MACRO * WC))

    in_maps = [{"X": Xdev[c], "W": Wdev[c]} for c in range(N_CORES)]
    return in_maps, (order, slot)


def _unshard(outs, meta):
    order, slot = meta
    stacked = np.stack(outs)  # [N_CORES, 128, N_GRP*B] fp16
    c = slot // SLOTS_CORE
    r = slot % SLOTS_CORE
    m = r // SPM
    b = (r % SPM) // B
    j = slot % B
    g = m // GROUP
    i = m % GROUP
    res = np.empty((M, 2), np.float32)
    res[order, 0] = stacked[c, 64 * i + b, g * B + j].astype(np.float32)
    res[order, 1] = stacked[c, 64 * i + 32 + b, g * B + j].astype(np.float32)
    return res


def kernel(tMP, tKF, kf_ids, mp_ids, idxKF, idxMP):
    from concourse.bass_utils import run_bass_kernel_spmd

    if "nc" not in _CACHE:
        _CACHE["nc"] = _build()
    nc = _CACHE["nc"]
    in_maps, meta = _prep_inputs(tMP, tKF, kf_ids, mp_ids, idxKF, idxMP)
    res = run_bass_kernel_spmd(nc, in_maps, core_ids=list(range(N_CORES)))
    outs = [res.results[i]["out"] for i in range(N_CORES)]
    return _unshard(outs, meta)
